# revision 1
# baseline (speedup 1.0000x reference)
"""Trainium2 Bass kernel for a 2-layer DGL-style RGCN + MLP head, sharded
across 8 NeuronCores.

Strategy (dst-node sharding, aggregate-first):
  - Core c owns destination nodes [c*D0/8, (c+1)*D0/8) of layer 0 and the
    analogous slice of layer 1. Edges are partitioned by destination on the
    host; relation-specific aggregation is computed as
        S[(r, d)] = sum_{e: dst=d, rel=r} x[src[e]]
    via on-device row gathers (indirect DMA) + one-hot scatter matmuls on
    the tensor engine, then h = concat_r(S_r) @ concat(W_r; Wloop).
  - BatchNorm statistics are partial per core and combined with a tiny
    AllReduce; layer-0 output is AllGathered (node-major) so every core can
    gather rows for its layer-1 edges. The MLP head runs on the core's own
    node slice; the host concatenates the 8 output shards.
All control flow is static: per-(window, relation) K-tile counts are the max
over cores (computed from the actual edge data at trace time), with padded
edges masked out of the one-hot matrices.
"""

import math
import os
from contextlib import ExitStack

import numpy as np

import concourse.bass as bass
import concourse.bacc as bacc
import concourse.mybir as mybir
import concourse.tile as tile
from concourse.bass_utils import run_bass_kernel_spmd

F32 = mybir.dt.float32
I32 = mybir.dt.int32
NCORES = 8
P = 128
EPS = 1e-5
NR = 5  # relations (excluding self-loop)


# ---------------------------------------------------------------- host prep

def _prep_layer(src, dst, etype, n_dst, n_src, self_loop_gather):
    """Partition edges by destination core, window them, compute shared
    K-tile caps, and emit per-core idx/bkt arrays in [128, KT_total] layout.

    Returns (idx_arrs, bkt_arrs, kt, d_core, nwin) where kt[w][r] is the
    K-tile count of (window w, relation r) shared by all cores; r == NR is
    the self-loop K-tile (present only when self_loop_gather).
    """
    d_core = n_dst // NCORES
    nwin = math.ceil(d_core / P)
    per_core = []
    for c in range(NCORES):
        m = (dst >= c * d_core) & (dst < (c + 1) * d_core)
        es, ed, er = src[m], dst[m] - c * d_core, etype[m]
        key = (ed // P) * NR + er  # (window, rel) group id
        order = np.argsort(key, kind="stable")
        per_core.append((es[order], ed[order], er[order], key[order]))

    # counts per (core, window, rel)
    cnt = np.zeros((NCORES, nwin, NR), np.int64)
    for c in range(NCORES):
        _, _, _, key = per_core[c]
        ids, n = np.unique(key, return_counts=True)
        cnt[c, ids // NR, ids % NR] = n
    cap = cnt.max(axis=0)  # [nwin, NR]
    kt = np.ceil(cap / P).astype(np.int64)  # K-tiles per (w, r)

    nsl = 1 if self_loop_gather else 0
    kt_tot = int(kt.sum()) + nsl * nwin
    idx_arrs, bkt_arrs = [], []
    for c in range(NCORES):
        es, ed, er, key = per_core[c]
        starts = np.zeros(nwin * NR + 1, np.int64)
        np.add.at(starts, key + 1, 1)
        starts = np.cumsum(starts)
        idx = np.zeros((kt_tot, P), np.int32)
        bkt = np.full((kt_tot, P), -1.0, np.float32)
        j = 0
        for w in range(nwin):
            for r in range(NR):
                nk = int(kt[w, r])
                if nk == 0:
                    continue
                g0, g1 = starts[w * NR + r], starts[w * NR + r + 1]
                n = g1 - g0
                flat_i = np.zeros(nk * P, np.int32)
                flat_b = np.full(nk * P, -1.0, np.float32)
                flat_i[:n] = es[g0:g1]
                flat_b[:n] = (ed[g0:g1] - w * P).astype(np.float32)
                idx[j:j + nk] = flat_i.reshape(nk, P)
                bkt[j:j + nk] = flat_b.reshape(nk, P)
                j += nk
            if self_loop_gather:
                nw = min(P, d_core - w * P)
                row = np.zeros(P, np.int32)
                row[:nw] = c * d_core + w * P + np.arange(nw, dtype=np.int32)
                idx[j] = row
                j += 1
        assert j == kt_tot
        idx_arrs.append(np.ascontiguousarray(idx.T))  # [128, KT_total]
        bkt_arrs.append(np.ascontiguousarray(bkt.T))
    return idx_arrs, bkt_arrs, kt, d_core, nwin


def _stack_wcat_rhs(W, Wl):
    """[12, 128, Dout] tiles: (r, kh) -> W[r][kh*128:(kh+1)*128, :]; r=5 -> Wl."""
    H = W.shape[1]
    tiles = []
    for r in range(NR + 1):
        M = Wl if r == NR else W[r]
        for kh in range(H // P):
            tiles.append(M[kh * P:(kh + 1) * P, :])
    return np.stack(tiles).astype(np.float32)


def _stack_wcat_lhsT(W, Wl):
    """[12, 2, 128, 128] tiles: [(r,kh)][ofh] = W[r][kh slice, ofh slice]."""
    H = W.shape[1]
    out = []
    for r in range(NR + 1):
        M = Wl if r == NR else W[r]
        for kh in range(H // P):
            out.append([M[kh * P:(kh + 1) * P, o * P:(o + 1) * P]
                        for o in range(H // P)])
    return np.asarray(out, np.float32)


# ---------------------------------------------------------------- program

def _build(shapes, kt0, kt1, nwin0, nwin1, d0c, d1c, n_src, n_d0, n_d1, C):
    H = 256
    KT0 = int(kt0.sum()) + nwin0  # + self-loop tile per window
    KT1 = int(kt1.sum()) + nwin1
    ktw_max = max(int(kt0.sum(axis=1).max()), int(kt1.sum(axis=1).max())) + 1

    nc = bacc.Bacc("TRN2", target_bir_lowering=False, debug=False,
                   num_devices=NCORES)
    t_x = nc.dram_tensor("x", [n_src, H], F32, kind="ExternalInput").ap()
    t_idx0 = nc.dram_tensor("idx0", [P, KT0], I32, kind="ExternalInput").ap()
    t_bkt0 = nc.dram_tensor("bkt0", [P, KT0], F32, kind="ExternalInput").ap()
    t_idx1 = nc.dram_tensor("idx1", [P, KT1], I32, kind="ExternalInput").ap()
    t_bkt1 = nc.dram_tensor("bkt1", [P, KT1], F32, kind="ExternalInput").ap()
    t_wc0 = nc.dram_tensor("wc0", [12, P, H], F32, kind="ExternalInput").ap()
    t_wc1 = nc.dram_tensor("wc1", [12, 2, P, P], F32, kind="ExternalInput").ap()
    t_wm1 = nc.dram_tensor("wm1", [2, 2, P, P], F32, kind="ExternalInput").ap()
    t_wm2 = nc.dram_tensor("wm2", [2, P, C], F32, kind="ExternalInput").ap()
    t_vec = nc.dram_tensor("vecs", [1, 2 * H + C], F32, kind="ExternalInput").ap()
    t_vp = nc.dram_tensor("vecp", [P, 8], F32, kind="ExternalInput").ap()
    t_out = nc.dram_tensor("out", [d1c, C], F32, kind="ExternalOutput").ap()
    debug = bool(os.environ.get("RGCN_KERNEL_DEBUG"))
    if debug:
        t_h0dbg = nc.dram_tensor("h0dbg", [d0c, H], F32,
                                 kind="ExternalOutput").ap()
        t_stdbg = nc.dram_tensor("stdbg", [1, 2 * H], F32,
                                 kind="ExternalOutput").ap()

    from concourse.masks import make_identity

    with tile.TileContext(nc) as tc, ExitStack() as ctx:
        const = ctx.enter_context(tc.tile_pool(name="const", bufs=1))
        sb = ctx.enter_context(tc.tile_pool(name="sb", bufs=2))
        stg_p = ctx.enter_context(tc.tile_pool(name="stg", bufs=2))
        oh_p = ctx.enter_context(tc.tile_pool(name="ohp", bufs=4))
        ps_s = ctx.enter_context(tc.tile_pool(name="ps_s", bufs=1, space="PSUM"))
        ps_h = ctx.enter_context(tc.tile_pool(name="ps_h", bufs=2, space="PSUM"))
        ps_c = ctx.enter_context(tc.tile_pool(name="ps_c", bufs=1, space="PSUM"))
        dram = ctx.enter_context(tc.tile_pool(name="dram", bufs=1, space="DRAM"))
        hold = ctx.enter_context(tc.tile_pool(name="hold", bufs=1))

        # ---- constants
        ident = const.tile([P, P], F32)
        make_identity(nc, ident[:])
        iota_i = const.tile([P, P], I32)
        nc.gpsimd.iota(iota_i[:], pattern=[[1, P]], base=0, channel_multiplier=0)
        iota_f = const.tile([P, P], F32)
        nc.vector.tensor_copy(iota_f[:], iota_i[:])
        ones_col = const.tile([P, 1], F32)
        nc.vector.memset(ones_col[:], 1.0)
        ones_row = const.tile([1, P], F32)
        nc.vector.memset(ones_row[:], 1.0)
        eps_t = const.tile([P, 1], F32)
        nc.vector.memset(eps_t[:], EPS)

        wc0 = const.tile([P, 12 * H], F32)
        nc.sync.dma_start(wc0[:].rearrange("p (t d) -> p t d", t=12),
                          t_wc0.rearrange("t p d -> p t d"))
        wc1 = const.tile([P, 24 * P], F32)
        nc.sync.dma_start(wc1[:].rearrange("p (t d) -> p t d", t=24),
                          t_wc1.rearrange("t o p d -> p (t o) d"))
        wm1 = const.tile([P, 4 * P], F32)
        nc.sync.dma_start(wm1[:].rearrange("p (t d) -> p t d", t=4),
                          t_wm1.rearrange("t o p d -> p (t o) d"))
        wm2 = const.tile([P, 2 * C], F32)
        nc.sync.dma_start(wm2[:].rearrange("p (t d) -> p t d", t=2),
                          t_wm2.rearrange("t p d -> p t d"))
        vecs = const.tile([1, 2 * H + C], F32)  # g0|be0|bm2 rows... packed
        nc.sync.dma_start(vecs[:], t_vec[:, :])
        vecp = const.tile([P, 8], F32)  # g1,be1,gm,bem as [128, ofh] pairs
        nc.sync.dma_start(vecp[:], t_vp[:, :])

        h0_loc = dram.tile([d0c, H], F32)
        h0_full = dram.tile([NCORES * d0c, H], F32, addr_space="Shared")

        # bm2 broadcast [P, C]
        bm2_ps = ps_c.tile([P, C], F32, space="PSUM", tag="bcast")
        nc.tensor.matmul(bm2_ps[:, :], lhsT=ones_row[:],
                         rhs=vecs[:, 2 * H:2 * H + C], start=True, stop=True)
        bm2b = const.tile([P, C], F32)
        nc.vector.tensor_copy(bm2b[:], bm2_ps[:, :])

        # ================= generic layer machinery =================
        def scatter_windows(w, nw, kt, t_idx, t_bkt, kt_off, src_t):
            """Gather + one-hot scatter for one window. Returns SBUF tiles
            s_sb[kh] of shape [128, 6*128] = S^T[(cf slice kh), (r, node)]."""
            ktw = int(kt[w].sum()) + 1  # incl self-loop K-tile
            idx_t = sb.tile([P, ktw], I32, tag="idx")
            nc.sync.dma_start(idx_t[:], t_idx[:, kt_off:kt_off + ktw])
            bkt_t = sb.tile([P, ktw], F32, tag="bkt")
            nc.sync.dma_start(bkt_t[:], t_bkt[:, kt_off:kt_off + ktw])
            stg = stg_p.tile([P, ktw_max * H], F32, tag="stg")
            # one indirect gather per K-tile: HW reads one row per partition
            # per call (multi-column index tiles are a sim-only extension)
            for g in range(ktw):
                nc.gpsimd.indirect_dma_start(
                    out=stg[:, g * H:(g + 1) * H], out_offset=None,
                    in_=src_t[:],
                    in_offset=bass.IndirectOffsetOnAxis(ap=idx_t[:, g:g + 1],
                                                        axis=0),
                )
            s_ps = [ps_s.tile([P, 6 * P], F32, space="PSUM", tag=f"s{kh}",
                              name=f"s_ps{kh}")
                    for kh in range(2)]
            t = 0
            for r in range(NR):
                nk = int(kt[w, r])
                for k in range(nk):
                    oh = oh_p.tile([P, P], F32, tag="oh")
                    nc.vector.tensor_tensor(
                        out=oh[:], in0=bkt_t[:, t:t + 1].to_broadcast([P, P]),
                        in1=iota_f[:], op=mybir.AluOpType.is_equal)
                    for kh in range(2):
                        nc.tensor.matmul(
                            s_ps[kh][:, r * P:(r + 1) * P],
                            lhsT=stg[:, t * H + kh * P: t * H + kh * P + P],
                            rhs=oh[:], start=(k == 0), stop=(k == nk - 1))
                    t += 1
            # self-loop K-tile: S[(5, d)] = row d  ->  rhs = identity
            for kh in range(2):
                nc.tensor.matmul(
                    s_ps[kh][:, NR * P:(NR + 1) * P],
                    lhsT=stg[:, t * H + kh * P: t * H + kh * P + P],
                    rhs=ident[:], start=True, stop=True)
            t += 1
            assert t == ktw
            s_sb = []
            for kh in range(2):
                s = sb.tile([P, 6 * P], F32, tag=f"ssb{kh}", name=f"ssb{kh}")
                nc.vector.tensor_copy(s[:], s_ps[kh][:])
                s_sb.append(s)
            return s_sb

        # ================= layer 0 =================
        hpre0 = hold.tile([P, nwin0 * H], F32)
        st0_ps = ps_c.tile([1, 2 * H], F32, space="PSUM", tag="st0")
        kt_off = 0
        for w in range(nwin0):
            nw = min(P, d0c - w * P)
            s_sb = scatter_windows(w, nw, kt0, t_idx0, t_bkt0, kt_off, t_x)
            kt_off += int(kt0[w].sum()) + 1
            hp = ps_h.tile([P, 512], F32, space="PSUM", tag="hp")
            for r in range(NR + 1):
                for kh in range(2):
                    nc.tensor.matmul(
                        hp[:nw, :H], lhsT=s_sb[kh][:, r * P:r * P + nw],
                        rhs=wc0[:, (r * 2 + kh) * H:(r * 2 + kh + 1) * H],
                        start=(r == 0 and kh == 0), stop=(r == NR and kh == 1))
            hs = hpre0[:, w * H:(w + 1) * H]
            nc.vector.tensor_copy(hs[:nw, :], hp[:nw, :H])
            # single accumulation group for [sum | sumsq]: two concurrent
            # groups in one PSUM bank are illegal (start zeroes the bank)
            cat = sb.tile([P, 2 * H], F32, tag="cat")
            nc.vector.tensor_copy(cat[:nw, :H], hs[:nw, :])
            nc.scalar.activation(cat[:nw, H:], hs[:nw, :],
                                 mybir.ActivationFunctionType.Square)
            nc.tensor.matmul(st0_ps[:, :], lhsT=ones_col[:nw, :],
                             rhs=cat[:nw, :], start=(w == 0),
                             stop=(w == nwin0 - 1))

        # ---- BN0 stats allreduce
        st0_sb = sb.tile([1, 2 * H], F32, tag="st0sb")
        nc.vector.tensor_copy(st0_sb[:], st0_ps[:])
        st0_in = dram.tile([1, 2 * H], F32)
        st0_out = dram.tile([1, 2 * H], F32, addr_space="Shared")
        nc.sync.dma_start(st0_in[:], st0_sb[:])
        nc.gpsimd.collective_compute(
            "AllReduce", mybir.AluOpType.add,
            replica_groups=[list(range(NCORES))],
            ins=[st0_in.opt()], outs=[st0_out.opt()])
        stg0 = sb.tile([1, 2 * H], F32, tag="stg0")
        nc.sync.dma_start(stg0[:], st0_out[:])

        # A = g/std, B = be - mean*A  (feat along free dim, 1 partition)
        nrm = sb.tile([1, 6 * H], F32, tag="nrm")
        mean, ex2, var, A, mA, B = (nrm[:, i * H:(i + 1) * H]
                                    for i in range(6))
        nc.vector.tensor_scalar_mul(mean, stg0[:, :H], 1.0 / n_d0)
        nc.vector.tensor_scalar_mul(ex2, stg0[:, H:], 1.0 / n_d0)
        nc.vector.tensor_tensor(out=var, in0=mean, in1=mean,
                                op=mybir.AluOpType.mult)
        nc.vector.tensor_tensor(out=var, in0=ex2, in1=var,
                                op=mybir.AluOpType.subtract)
        nc.scalar.activation(var, var, mybir.ActivationFunctionType.Sqrt,
                             bias=eps_t[:1, :])
        nc.vector.reciprocal(var, var)  # var now holds 1/std
        nc.vector.tensor_tensor(out=A, in0=vecs[:, :H], in1=var,
                                op=mybir.AluOpType.mult)
        nc.vector.tensor_tensor(out=mA, in0=mean, in1=A,
                                op=mybir.AluOpType.mult)
        nc.vector.tensor_tensor(out=B, in0=vecs[:, H:2 * H], in1=mA,
                                op=mybir.AluOpType.subtract)
        ab_ps = ps_c.tile([P, 2 * H], F32, space="PSUM", tag="bcast")
        nc.tensor.matmul(ab_ps[:, :H], lhsT=ones_row[:], rhs=A,
                         start=True, stop=True)
        nc.tensor.matmul(ab_ps[:, H:2 * H], lhsT=ones_row[:], rhs=B,
                         start=True, stop=True)
        ab = sb.tile([P, 2 * H], F32, tag="ab")
        nc.vector.tensor_copy(ab[:], ab_ps[:, :2 * H])

        # ---- BN0 apply + ELU + store h0
        for w in range(nwin0):
            nw = min(P, d0c - w * P)
            hs = hpre0[:nw, w * H:(w + 1) * H]
            y = sb.tile([P, H], F32, tag="y")
            nc.vector.tensor_tensor(out=y[:nw, :], in0=hs, in1=ab[:nw, :H],
                                    op=mybir.AluOpType.mult)
            nc.vector.tensor_tensor(out=y[:nw, :], in0=y[:nw, :],
                                    in1=ab[:nw, H:], op=mybir.AluOpType.add)
            e = sb.tile([P, H], F32, tag="e")
            nc.scalar.activation(e[:nw, :], y[:nw, :],
                                 mybir.ActivationFunctionType.Exp)
            nc.vector.tensor_scalar(out=e[:nw, :], in0=e[:nw, :], scalar1=1.0,
                                    scalar2=-1.0, op0=mybir.AluOpType.min,
                                    op1=mybir.AluOpType.add)
            nc.scalar.activation(y[:nw, :], y[:nw, :],
                                 mybir.ActivationFunctionType.Relu)
            nc.vector.tensor_tensor(out=y[:nw, :], in0=y[:nw, :], in1=e[:nw, :],
                                    op=mybir.AluOpType.add)
            nc.sync.dma_start(h0_loc[w * P:w * P + nw, :], y[:nw, :])

        if debug:
            nc.gpsimd.dma_start(t_h0dbg[:, :], h0_loc[:, :])
            nc.gpsimd.dma_start(t_stdbg[:, :], st0_out[:, :])
        # ---- AllGather h0
        nc.gpsimd.collective_compute(
            "AllGather", mybir.AluOpType.bypass,
            replica_groups=[list(range(NCORES))],
            ins=[h0_loc.opt()], outs=[h0_full.opt()])

        # ================= layer 1 (feat-major outputs) =================
        h1T = [hold.tile([P, d1c], F32, name=f"h1T{o}") for o in range(2)]
        kt_off = 0
        for w in range(nwin1):
            nw = min(P, d1c - w * P)
            s_sb = scatter_windows(w, nw, kt1, t_idx1, t_bkt1, kt_off, h0_full)
            kt_off += int(kt1[w].sum()) + 1
            for o in range(2):
                zp = ps_h.tile([P, P], F32, space="PSUM", tag="hp")
                for r in range(NR + 1):
                    for kh in range(2):
                        ti = r * 2 + kh
                        nc.tensor.matmul(
                            zp[:, :nw],
                            lhsT=wc1[:, (ti * 2 + o) * P:(ti * 2 + o + 1) * P],
                            rhs=s_sb[kh][:, r * P:r * P + nw],
                            start=(r == 0 and kh == 0),
                            stop=(r == NR and kh == 1))
                nc.vector.tensor_copy(h1T[o][:, w * P:w * P + nw], zp[:, :nw])

        # ---- BN1 (feat on partitions)
        def bn_feat(hT, n_nodes, gvec_col, bvec_col, st_tag):
            """BN stats over free dim for feat-major tiles; returns A,B [P,1]
            per ofh after AllReduce. hT: list of 2 tiles [P, n_nodes]."""
            st = sb.tile([P, 4], F32, tag=f"{st_tag}sb")
            for o in range(2):
                nc.vector.tensor_reduce(st[:, o:o + 1], hT[o][:, :n_nodes],
                                        axis=mybir.AxisListType.X,
                                        op=mybir.AluOpType.add)
                sq = sb.tile([P, d1c], F32, tag="sqT", bufs=1)
                nc.scalar.activation(sq[:, :n_nodes], hT[o][:, :n_nodes],
                                     mybir.ActivationFunctionType.Square,
                                     accum_out=st[:, 2 + o:3 + o])
            st_in = dram.tile([P, 4], F32, name=f"{st_tag}_in")
            st_out = dram.tile([P, 4], F32, addr_space="Shared",
                               name=f"{st_tag}_out")
            nc.sync.dma_start(st_in[:], st[:])
            nc.gpsimd.collective_compute(
                "AllReduce", mybir.AluOpType.add,
                replica_groups=[list(range(NCORES))],
                ins=[st_in.opt()], outs=[st_out.opt()])
            stg_ = sb.tile([P, 4], F32, tag=f"{st_tag}g")
            nc.sync.dma_start(stg_[:], st_out[:])
            abv = sb.tile([P, 8], F32, tag=f"{st_tag}ab")
            n_tot = n_nodes * NCORES
            for o in range(2):
                mean_ = abv[:, o:o + 1]
                var_ = abv[:, 2 + o:3 + o]
                A_ = abv[:, 4 + o:5 + o]
                B_ = abv[:, 6 + o:7 + o]
                nc.vector.tensor_scalar_mul(mean_, stg_[:, o:o + 1], 1.0 / n_tot)
                nc.vector.tensor_scalar_mul(var_, stg_[:, 2 + o:3 + o], 1.0 / n_tot)
                tmp = sb.tile([P, 1], F32, tag=f"{st_tag}t")
                nc.vector.tensor_tensor(out=tmp[:], in0=mean_, in1=mean_,
                                        op=mybir.AluOpType.mult)
                nc.vector.tensor_tensor(out=var_, in0=var_, in1=tmp[:],
                                        op=mybir.AluOpType.subtract)
                nc.scalar.activation(var_, var_,
                                     mybir.ActivationFunctionType.Sqrt,
                                     bias=eps_t[:, :])
                nc.vector.reciprocal(var_, var_)
                nc.vector.tensor_tensor(out=A_, in0=gvec_col(o), in1=var_,
                                        op=mybir.AluOpType.mult)
                nc.vector.tensor_tensor(out=tmp[:], in0=mean_, in1=A_,
                                        op=mybir.AluOpType.mult)
                nc.vector.tensor_tensor(out=B_, in0=bvec_col(o), in1=tmp[:],
                                        op=mybir.AluOpType.subtract)
            return abv

        ab1 = bn_feat(h1T, d1c, lambda o: vecp[:, o:o + 1],
                      lambda o: vecp[:, 2 + o:3 + o], "st1")
        for o in range(2):
            nc.vector.tensor_scalar(out=h1T[o][:], in0=h1T[o][:],
                                    scalar1=ab1[:, 4 + o:5 + o],
                                    scalar2=ab1[:, 6 + o:7 + o],
                                    op0=mybir.AluOpType.mult,
                                    op1=mybir.AluOpType.add)
            e = sb.tile([P, d1c], F32, tag="eT", bufs=1)
            nc.scalar.activation(e[:], h1T[o][:],
                                 mybir.ActivationFunctionType.Exp)
            nc.vector.tensor_scalar(out=e[:], in0=e[:], scalar1=1.0,
                                    scalar2=-1.0, op0=mybir.AluOpType.min,
                                    op1=mybir.AluOpType.add)
            nc.scalar.activation(h1T[o][:], h1T[o][:],
                                 mybir.ActivationFunctionType.Relu)
            nc.vector.tensor_tensor(out=h1T[o][:], in0=h1T[o][:], in1=e[:],
                                    op=mybir.AluOpType.add)

        # ---- head: z = h1 @ Wm1 (feat-major), BN, ReLU
        z2T = [hold.tile([P, d1c], F32, name=f"z2T{o}") for o in range(2)]
        nwz = math.ceil(d1c / 512)
        for o in range(2):
            for wz in range(nwz):
                n = min(512, d1c - wz * 512)
                zp = ps_h.tile([P, 512], F32, space="PSUM", tag="hp")
                for kh in range(2):
                    nc.tensor.matmul(
                        zp[:, :n],
                        lhsT=wm1[:, (kh * 2 + o) * P:(kh * 2 + o + 1) * P],
                        rhs=h1T[kh][:, wz * 512:wz * 512 + n],
                        start=(kh == 0), stop=(kh == 1))
                nc.vector.tensor_copy(z2T[o][:, wz * 512:wz * 512 + n],
                                      zp[:, :n])
        ab2 = bn_feat(z2T, d1c, lambda o: vecp[:, 4 + o:5 + o],
                      lambda o: vecp[:, 6 + o:7 + o], "st2")
        for o in range(2):
            nc.vector.tensor_scalar(out=z2T[o][:], in0=z2T[o][:],
                                    scalar1=ab2[:, 4 + o:5 + o],
                                    scalar2=ab2[:, 6 + o:7 + o],
                                    op0=mybir.AluOpType.mult,
                                    op1=mybir.AluOpType.add)
            nc.scalar.activation(z2T[o][:], z2T[o][:],
                                 mybir.ActivationFunctionType.Relu)

        # ---- out = y2 @ Wm2 + bm2  (node-major output chunks)
        for w in range(nwin1):
            nw = min(P, d1c - w * P)
            op_ = ps_h.tile([P, C], F32, space="PSUM", tag="hp")
            for kh in range(2):
                nc.tensor.matmul(op_[:nw, :], lhsT=z2T[kh][:, w * P:w * P + nw],
                                 rhs=wm2[:, kh * C:(kh + 1) * C],
                                 start=(kh == 0), stop=(kh == 1))
            ob = sb.tile([P, C], F32, tag="ob")
            nc.vector.tensor_tensor(out=ob[:nw, :], in0=op_[:nw, :],
                                    in1=bm2b[:nw, :], op=mybir.AluOpType.add)
            nc.sync.dma_start(t_out[w * P:w * P + nw, :], ob[:nw, :])

    nc.compile()
    return nc


# ---------------------------------------------------------------- entry

def kernel(x, src0, dst0, etype0, src1, dst1, etype1, n_dst0, n_dst1,
           W0, Wl0, b0, g0, be0, W1, Wl1, b1, g1, be1,
           Wm1, bm1, gm, bem, Wm2, bm2):
    x = np.asarray(x, np.float32)
    n_src, H = x.shape
    n_d0, n_d1 = int(n_dst0), int(n_dst1)
    C = np.asarray(Wm2).shape[1]

    src0 = np.asarray(src0, np.int64)
    dst0 = np.asarray(dst0, np.int64)
    etype0 = np.asarray(etype0, np.int64)
    src1 = np.asarray(src1, np.int64)
    dst1 = np.asarray(dst1, np.int64)
    etype1 = np.asarray(etype1, np.int64)

    idx0, bkt0, kt0, d0c, nwin0 = _prep_layer(src0, dst0, etype0, n_d0, n_src, True)
    idx1, bkt1, kt1, d1c, nwin1 = _prep_layer(src1, dst1, etype1, n_d1, n_d0, True)
    # note: biases b0/b1/bm1 cancel inside BatchNorm (training mode); only
    # bm2 reaches the output.
    wc0 = _stack_wcat_rhs(np.asarray(W0, np.float32), np.asarray(Wl0, np.float32))
    wc1 = _stack_wcat_lhsT(np.asarray(W1, np.float32), np.asarray(Wl1, np.float32))
    wm1 = np.stack([
        [np.asarray(Wm1, np.float32)[kh * P:(kh + 1) * P, o * P:(o + 1) * P]
         for o in range(2)] for kh in range(2)])
    wm2 = np.stack([np.asarray(Wm2, np.float32)[kh * P:(kh + 1) * P, :]
                    for kh in range(2)])
    vecs = np.concatenate([np.asarray(g0, np.float32),
                           np.asarray(be0, np.float32),
                           np.asarray(bm2, np.float32)])[None, :]
    # vecp[p, 2*i+o] -> vec_i[o*128+p]; columns: g1 (0,1), be1 (2,3), gm (4,5), bem (6,7)
    vecp = np.ascontiguousarray(
        np.stack([np.asarray(v, np.float32).reshape(2, P).T
                  for v in (g1, be1, gm, bem)], axis=1).reshape(P, 8))

    nc = _build((n_src, H), kt0, kt1, nwin0, nwin1, d0c, d1c,
                n_src, n_d0, n_d1, C)
    in_maps = []
    for c in range(NCORES):
        in_maps.append({
            "x": x, "idx0": idx0[c], "bkt0": bkt0[c],
            "idx1": idx1[c], "bkt1": bkt1[c],
            "wc0": wc0, "wc1": wc1, "wm1": wm1, "wm2": wm2,
            "vecs": vecs, "vecp": vecp,
        })
    trace = bool(os.environ.get("RGCN_KERNEL_PROFILE"))
    kw = {}
    if trace:
        _install_ntff_hook()
        kw = dict(trace=True, tmpdir=os.environ.get("RGCN_KERNEL_TMPDIR"))
    res = run_bass_kernel_spmd(nc, in_maps, list(range(NCORES)), **kw)
    if trace:
        LAST_EXEC_TIME_NS[0] = res.exec_time_ns
        print(f"[kernel] exec_time_ns: {res.exec_time_ns}")
    out = np.concatenate([res.results[c]["out"] for c in range(NCORES)], axis=0)
    return out.astype(np.float32)


LAST_EXEC_TIME_NS = [None]


def _install_ntff_hook():
    """Best-effort NTFF profiling under axon (the image's antenv lacks
    axon_hooks; register the ctypes hook ourselves). No-op on failure."""
    import sys, types
    try:
        import antenv
        if "antenv.axon_hooks" not in sys.modules:
            hooks = types.ModuleType("antenv.axon_hooks")
            _h = [None]
            hooks.set_axon_ntff_profile_hook = lambda h: _h.__setitem__(0, h)
            hooks.get_axon_ntff_profile_hook = lambda: _h[0]
            sys.modules["antenv.axon_hooks"] = hooks
            antenv.axon_hooks = hooks
            from trn_agent_boot.trn_boot import _ntff_profile_via_ctypes
            hook = _ntff_profile_via_ctypes("/opt/axon/libaxon_pjrt.so")
            if hook is not None:
                hooks.set_axon_ntff_profile_hook(hook)
        import concourse.bass_utils as bu
        bu.upload_artifacts = lambda tmpdir: "local://" + tmpdir
    except Exception as e:  # profiling is optional
        print(f"[kernel] profiling hook unavailable: {e}")

